# revision 13
# baseline (speedup 1.0000x reference)
"""Trainium2 Bass kernel for the sparse_attention nn.Module problem.

Reference computation (B=4, H=W=64, C=128, HEADS=4, DIM_HEAD=32):
  qkv = x @ w_qkv ; q,k = l2norm over token axis ; sim = q@k^T * 10
  attn = softmax(sim) ; out = (attn @ v) @ w_out + b_out

Sharding: 8 cores = (batch b, query-half). Each core computes attention for
2048 query rows of one batch image against all 4096 keys, all 4 heads.
The token axis of each core's input is pre-rotated on the host so that the
core's queries are always tokens [0, 2048) -> all 8 cores run ONE program.

Device dataflow (per core), everything kept transposed ([feature, token]):
  qT/kT = w^T @ xT (PE), v natural = xT-chunks^T @ w_v (PE)
  gamma_d = 1/(||q_d|| * ||k_d||)  (DVE sumsq via tensor_tensor_reduce,
            ACT sqrt, DVE reciprocal), folded into qTs = qT[:, :2048]*gamma
  simT[j,i] (per head, row-packed 4x on PE via tile_position) -> PSUM
  exp = ACT Exp(10*simT) PSUM->SBUF  (softmax max-subtraction is skipped:
        |10*sim| <= ~0.15 for l2-normalized q,k, exactly representable)
  numerator^T[d,i] += V_h^T-chunk @ exp  (PE, col-packed 4 heads into one
        PSUM bank, accumulated over j-chunks)
  denom_h[i]      += ones^T @ exp        (PE, col-packed M=1 rows)
  outT = numer * recip(denom) (DVE recip + GPSIMD partition_broadcast + DVE)
  out_cT = w_out^T @ outT + b_out  (PE + DVE per-partition bias add)
Output is returned c-major [128, 2048]; host transposes and reassembles.
"""

import sys
from contextlib import ExitStack

import numpy as np

for _p in ("/opt/trn_rl_repo",):
    if _p not in sys.path:
        sys.path.insert(0, _p)

import concourse.bass as bass
import concourse.tile as tile
from concourse import bacc, mybir
from concourse._compat import with_exitstack

F32 = mybir.dt.float32
AF = mybir.ActivationFunctionType
ALU = mybir.AluOpType

S = 4096          # tokens per image
C = 128           # channels
NQ = 2048         # queries per core
HEADS = 4
DH = 32
SCALE = 10.0
N_CORES = 8

JC = S // 128     # 32 key chunks of 128
IC = NQ // 512    # 4 query chunks of 512


@with_exitstack
def _attention_kernel(ctx: ExitStack, tc: tile.TileContext):
    nc = tc.nc
    xT_d = nc.dram_tensor("xT", [C, S], F32, kind="ExternalInput").ap()
    wqkv_d = nc.dram_tensor("w_qkv", [C, 384], F32, kind="ExternalInput").ap()
    wout_d = nc.dram_tensor("w_out", [C, C], F32, kind="ExternalInput").ap()
    bout_d = nc.dram_tensor("b_out", [C, 1], F32, kind="ExternalInput").ap()
    out_d = nc.dram_tensor("out_cT", [C, NQ], F32, kind="ExternalOutput").ap()
    # internal DRAM bounce buffer for the per-head denominator reciprocal rows
    # (SBUF->SBUF partition-broadcast DMA is not supported; DRAM-source
    # partition-broadcast is)
    recd = nc.dram_tensor("rec_dram", [IC, HEADS, 512], F32).ap()

    consts = ctx.enter_context(tc.tile_pool(name="consts", bufs=1))
    big = ctx.enter_context(tc.tile_pool(name="big", bufs=1))
    expp = ctx.enter_context(tc.tile_pool(name="expp", bufs=3))
    recp = ctx.enter_context(tc.tile_pool(name="recp", bufs=2))
    psum = ctx.enter_context(tc.tile_pool(name="psum", bufs=3, space="PSUM"))
    psum_acc = ctx.enter_context(tc.tile_pool(name="psum_acc", bufs=1, space="PSUM"))

    # ---- load inputs ----
    wq = consts.tile([C, 384], F32)
    nc.sync.dma_start(out=wq[:], in_=wqkv_d)
    wo = consts.tile([C, C], F32)
    nc.sync.dma_start(out=wo[:], in_=wout_d)
    bias = consts.tile([C, 1], F32)
    nc.sync.dma_start(out=bias[:], in_=bout_d)
    ones = consts.tile([C, 1], F32)
    nc.vector.memset(ones[:], 1.0)
    zz = consts.tile([1, 512], F32)
    nc.vector.memset(zz[:], 0.0)
    xT = big.tile([C, S], F32)
    nc.sync.dma_start(out=xT[:], in_=xT_d)

    # ---- q/k projections (transposed layout: [feature, token]) ----
    qT = big.tile([C, S], F32)
    kT = big.tile([C, S], F32)
    for t in range(S // 512):
        pq = psum.tile([128, 512], F32, tag="st")
        nc.tensor.matmul(pq[:, 0:512], wq[:, 0:128], xT[:, 512 * t:512 * t + 512],
                         start=True, stop=True)
        nc.vector.tensor_copy(qT[:, 512 * t:512 * t + 512], pq[:, 0:512])
        pk = psum.tile([128, 512], F32, tag="st")
        nc.tensor.matmul(pk[:, 0:512], wq[:, 128:256], xT[:, 512 * t:512 * t + 512],
                         start=True, stop=True)
        nc.vector.tensor_copy(kT[:, 512 * t:512 * t + 512], pk[:, 0:512])

    # ---- v projection (natural layout chunks: v[token128, hd] per chunk) ----
    v = big.tile([C, S], F32)  # col block t holds v rows [128t, 128t+128)
    for t in range(JC):
        pv = psum.tile([128, 512], F32, tag="st")
        nc.tensor.matmul(pv[:, 0:128], xT[:, 128 * t:128 * t + 128], wq[:, 256:384],
                         start=True, stop=True)
        nc.vector.tensor_copy(v[:, 128 * t:128 * t + 128], pv[:, 0:128])

    # ---- norms: gamma = 1/sqrt(sumsq(q_d) * sumsq(k_d)), fold into qTs ----
    scratch = big.tile([C, S], F32)
    ssq = consts.tile([C, 2], F32)
    nc.scalar.activation(scratch[:], qT[:], AF.Square, accum_out=ssq[:, 0:1])
    nc.scalar.activation(scratch[:], kT[:], AF.Square, accum_out=ssq[:, 1:2])
    gam = consts.tile([C, 2], F32)
    nc.vector.tensor_mul(gam[:, 0:1], ssq[:, 0:1], ssq[:, 1:2])
    nc.scalar.sqrt(gam[:, 1:2], gam[:, 0:1])
    nc.vector.reciprocal(gam[:, 0:1], gam[:, 1:2])
    qTs = big.tile([C, NQ], F32)
    nc.vector.tensor_scalar_mul(qTs[:], qT[:, 0:NQ], gam[:, 0:1])

    # ---- main attention loop ----
    outT = big.tile([C, NQ], F32)
    for ic in range(IC):
        i0 = 512 * ic
        ppv = psum_acc.tile([128, 512], F32, tag="pv")
        pden = psum_acc.tile([128, 512], F32, tag="den")
        # zero-init both accumulator banks: one start=True matmul writing the
        # full bank (sets every has_written bit); all later matmuls accumulate.
        # (start=True clears has_written for the WHOLE bank, so the 4 packed
        # per-head groups sharing a bank must not each issue their own start.)
        nc.tensor.matmul(ppv[:, :], zz[0:1, 0:128], zz[0:1, 0:512],
                         start=True, stop=False, skip_group_check=True)
        nc.tensor.matmul(pden[:, :], zz[0:1, 0:128], zz[0:1, 0:512],
                         start=True, stop=False, skip_group_check=True)
        for jc in range(JC):
            j0 = 128 * jc
            exps = []
            for pair in range(2):  # heads (0,1) then (2,3)
                st = psum.tile([128, 1024], F32, tag="st")
                ex = expp.tile([128, 1024], F32, tag="ex")
                for hh in range(2):
                    h = 2 * pair + hh
                    hp = 32 * h
                    nc.tensor.matmul(
                        st[:, 512 * hh:512 * hh + 512],
                        kT[hp:hp + 32, j0:j0 + 128],
                        qTs[hp:hp + 32, i0:i0 + 512],
                        start=True, stop=True, tile_position=(hp, 0))
                nc.scalar.activation(ex[:], st[:], AF.Exp, scale=SCALE)
                exps.append(ex)
            for h in range(HEADS):
                hp = 32 * h
                ex = exps[h // 2][:, 512 * (h % 2):512 * (h % 2) + 512]
                nc.tensor.matmul(
                    ppv[hp:hp + 32, :],
                    v[:, j0 + hp:j0 + hp + 32],
                    ex,
                    start=False, stop=(jc == JC - 1), tile_position=(0, hp),
                    skip_group_check=True)
                nc.tensor.matmul(
                    pden[hp:hp + 1, :],
                    ones[:, 0:1],
                    ex,
                    start=False, stop=(jc == JC - 1), tile_position=(0, hp),
                    skip_group_check=True)
        # normalize: outT[hd, i] = ppv / den_h
        rec = recp.tile([128, 512], F32, tag="rec")
        recb = recp.tile([128, 512], F32, tag="recb")
        for h in range(HEADS):
            hp = 32 * h
            nc.vector.reciprocal(rec[hp:hp + 1, :], pden[hp:hp + 1, :])
            nc.sync.dma_start(out=recd[ic, h, :], in_=rec[hp:hp + 1, :])
            src = recd[ic, h, :]
            bcast = bass.AP(tensor=src.tensor, offset=src.offset,
                            ap=[[0, 32]] + list(src.ap))
            nc.sync.dma_start(out=recb[hp:hp + 32, :], in_=bcast)
            nc.vector.tensor_mul(outT[hp:hp + 32, i0:i0 + 512],
                                 ppv[hp:hp + 32, :], recb[hp:hp + 32, :])

    # ---- output projection: out_cT = w_out^T @ outT + b_out ----
    res = big.tile([C, NQ], F32)
    for t in range(IC):
        po = psum.tile([128, 512], F32, tag="st")
        nc.tensor.matmul(po[:, 0:512], wo[:], outT[:, 512 * t:512 * t + 512],
                         start=True, stop=True)
        nc.vector.tensor_scalar_add(res[:, 512 * t:512 * t + 512], po[:, 0:512],
                                    bias[:, 0:1])
    nc.sync.dma_start(out=out_d, in_=res[:])


_CACHE = {}


def build_program():
    if "nc" not in _CACHE:
        nc = bacc.Bacc("TRN2", debug=False, target_bir_lowering=False,
                       num_devices=N_CORES)
        with tile.TileContext(nc) as tc:
            _attention_kernel(tc)
        nc.compile()
        _CACHE["nc"] = nc
    return _CACHE["nc"]


def make_in_maps(x, w_qkv, w_out, b_out):
    in_maps = []
    for core in range(N_CORES):
        b, half = core // 2, core % 2
        i0 = half * NQ
        xr = np.asarray(x[b], dtype=np.float32).reshape(S, C)
        xT = np.ascontiguousarray(np.roll(xr, -i0, axis=0).T)
        in_maps.append({
            "xT": xT,
            "w_qkv": np.ascontiguousarray(w_qkv, dtype=np.float32),
            "w_out": np.ascontiguousarray(w_out, dtype=np.float32),
            "b_out": np.ascontiguousarray(b_out, dtype=np.float32).reshape(C, 1),
        })
    return in_maps


def assemble_output(per_core_outs):
    out = np.zeros((4, S, C), dtype=np.float32)
    for core, r in enumerate(per_core_outs):
        b, half = core // 2, core % 2
        out[b, half * NQ:(half + 1) * NQ] = np.asarray(r, dtype=np.float32).T
    return out.reshape(4, 64, 64, C)


def kernel(x, w_qkv, w_out, b_out):
    from concourse.bass_utils import run_bass_kernel_spmd
    nc = build_program()
    in_maps = make_in_maps(x, w_qkv, w_out, b_out)
    res = run_bass_kernel_spmd(nc, in_maps, list(range(N_CORES)))
    return assemble_output([r["out_cT"] for r in res.results])


if __name__ == "__main__":
    x = np.random.randn(4, 64, 64, C).astype(np.float32)
    w_qkv = (np.random.randn(C, 384) / np.sqrt(C)).astype(np.float32)
    w_out = (np.random.randn(C, C) / np.sqrt(C)).astype(np.float32)
    b_out = np.zeros(C, dtype=np.float32)
    out = kernel(x=x, w_qkv=w_qkv, w_out=w_out, b_out=b_out)
    print("kernel output", out.shape, out.dtype)


# revision 29
# speedup vs baseline: 1.2750x; 1.2750x over previous
"""Trainium2 Bass kernel for the sparse_attention nn.Module problem.

Reference computation (B=4, H=W=64, C=128, HEADS=4, DIM_HEAD=32):
  qkv = x @ w_qkv ; q,k = l2norm over token axis ; sim = q@k^T * 10
  attn = softmax(sim) ; out = (attn @ v) @ w_out + b_out

Sharding: 8 cores = (batch b, query-half). Each core computes attention for
2048 query rows of one batch image against all 4096 keys, all 4 heads.
The token axis of each core's input is pre-rotated on the host so that the
core's queries are always tokens [0, 2048) -> all 8 cores run ONE program.

Device dataflow (per core), everything kept transposed ([feature, token]):
  qT/kT = w^T @ xT (PE), v natural = xT-chunks^T @ w_v (PE)
  gamma_d = 1/(||q_d|| * ||k_d||)  (DVE sumsq via tensor_tensor_reduce,
            ACT sqrt, DVE reciprocal), folded into qTs = qT[:, :2048]*gamma
  simT[j,i] (per head, row-packed 4x on PE via tile_position) -> PSUM
  exp = ACT Exp(10*simT) PSUM->SBUF  (softmax max-subtraction is skipped:
        |10*sim| <= ~0.15 for l2-normalized q,k, exactly representable)
  numerator^T[d,i] += V_h^T-chunk @ exp  (PE, col-packed 4 heads into one
        PSUM bank, accumulated over j-chunks)
  denom_h[i]      += ones^T @ exp        (PE, col-packed M=1 rows)
  outT = numer * recip(denom) (DVE recip + GPSIMD partition_broadcast + DVE)
  out_cT = w_out^T @ outT + b_out  (PE + DVE per-partition bias add)
Output is returned c-major [128, 2048]; host transposes and reassembles.
"""

import sys
from contextlib import ExitStack

import ml_dtypes
import numpy as np

for _p in ("/opt/trn_rl_repo",):
    if _p not in sys.path:
        sys.path.insert(0, _p)

import concourse.bass as bass
import concourse.tile as tile
from concourse import bacc, mybir
from concourse._compat import with_exitstack

F32 = mybir.dt.float32
F32R = mybir.dt.float32r  # fp32 data, single-pass matmul (full rate at N>=256)
BF16 = mybir.dt.bfloat16
AF = mybir.ActivationFunctionType
ALU = mybir.AluOpType


S = 4096          # tokens per image
C = 128           # channels
NQ = 2048         # queries per core
HEADS = 4
DH = 32
SCALE = 10.0
N_CORES = 8

JC = S // 128     # 32 key chunks of 128
IC = NQ // 512    # 4 query chunks of 512


@with_exitstack
def _attention_kernel(ctx: ExitStack, tc: tile.TileContext):
    nc = tc.nc
    xT_d = nc.dram_tensor("xT", [C, S], F32R, kind="ExternalInput").ap()
    wqkv_d = nc.dram_tensor("w_qkv", [C, 384], F32R, kind="ExternalInput").ap()
    wout_d = nc.dram_tensor("w_out", [C, C], F32R, kind="ExternalInput").ap()
    bout_d = nc.dram_tensor("b_out", [C, 1], F32, kind="ExternalInput").ap()
    out_d = nc.dram_tensor("out_cT", [C, NQ], F32, kind="ExternalOutput").ap()
    # internal DRAM bounce buffer for the per-head denominator reciprocal rows
    # (SBUF->SBUF partition-broadcast DMA is not supported; DRAM-source
    # partition-broadcast is)
    recd = nc.dram_tensor("rec_dram", [IC, HEADS, 512], F32).ap()

    consts = ctx.enter_context(tc.tile_pool(name="consts", bufs=1))
    big = ctx.enter_context(tc.tile_pool(name="big", bufs=1))
    expp = ctx.enter_context(tc.tile_pool(name="expp", bufs=3))
    recp = ctx.enter_context(tc.tile_pool(name="recp", bufs=2))
    psum = ctx.enter_context(tc.tile_pool(name="psum", bufs=2, space="PSUM"))
    psum_acc = ctx.enter_context(tc.tile_pool(name="psum_acc", bufs=4, space="PSUM"))

    # ---- load inputs ----
    wq = consts.tile([C, 384], F32R)
    nc.sync.dma_start(out=wq[:], in_=wqkv_d)
    wo = consts.tile([C, C], F32R)
    nc.sync.dma_start(out=wo[:], in_=wout_d)
    bias = consts.tile([C, 1], F32)
    nc.sync.dma_start(out=bias[:], in_=bout_d)
    xT = big.tile([C, S], F32R)
    nc.sync.dma_start(out=xT[:], in_=xT_d)

    # ---- q/k projections (transposed layout: [feature, token]) ----
    qT = big.tile([C, S], F32)
    kT = big.tile([C, S], F32R)
    for t in range(S // 512):
        pq = psum.tile([128, 512], F32, tag="st")
        nc.tensor.matmul(pq[:, 0:512], wq[:, 0:128],
                         xT[:, 512 * t:512 * t + 512], start=True, stop=True)
        nc.vector.tensor_copy(qT[:, 512 * t:512 * t + 512], pq[:, 0:512])
        pk = psum.tile([128, 512], F32, tag="st")
        nc.tensor.matmul(pk[:, 0:512], wq[:, 128:256],
                         xT[:, 512 * t:512 * t + 512], start=True, stop=True)
        nc.vector.tensor_copy(kT[:, 512 * t:512 * t + 512], pk[:, 0:512])

    # ---- v projection scattered into augmented PV weights ----
    # v_aug block blk=(h*JC+jc) is a [128,128] lhsT: out rows 32h..32h+32 get
    # head h's numerator, row (32h+32)%128 gets the softmax denominator.
    v_aug = big.tile([C, HEADS * JC * 128], F32R)
    va_u32 = v_aug[:].bitcast(mybir.dt.uint32)
    nc.vector.memset(va_u32, 0)
    for h in range(HEADS):
        onescol = (32 * h + 32) % 128
        view = v_aug[:, h * JC * 128:(h + 1) * JC * 128].rearrange(
            "p (b c) -> p b c", c=128)[:, :, onescol:onescol + 1]
        nc.vector.memset(view.bitcast(mybir.dt.uint32), 0x3F800000)
    for t in range(JC):
        pv = psum.tile([128, 512], F32, tag="st")
        nc.tensor.matmul(pv[:, 0:128], xT[:, 128 * t:128 * t + 128], wq[:, 256:384],
                         start=True, stop=True)
        for h in range(HEADS):
            hp = 32 * h
            nc.vector.tensor_copy(
                v_aug[:, (h * JC + t) * 128 + hp:(h * JC + t) * 128 + hp + 32],
                pv[:, hp:hp + 32])
    res = big.tile([C, NQ], F32)

    # ---- norms: gamma = 1/sqrt(sumsq(q_d) * sumsq(k_d)), fold into qTs ----
    scratch = big.tile([C, S], F32)
    ssq = consts.tile([C, 2], F32)
    nc.scalar.activation(scratch[:], qT[:], AF.Square, accum_out=ssq[:, 0:1])
    nc.scalar.activation(scratch[:], kT[:].bitcast(F32), AF.Square, accum_out=ssq[:, 1:2])
    gam = consts.tile([C, 2], F32)
    nc.vector.tensor_mul(gam[:, 0:1], ssq[:, 0:1], ssq[:, 1:2])
    nc.scalar.sqrt(gam[:, 1:2], gam[:, 0:1])
    nc.vector.reciprocal(gam[:, 0:1], gam[:, 1:2])
    qTs = big.tile([C, NQ], F32R)
    nc.vector.tensor_scalar_mul(qTs[:], qT[:, 0:NQ], gam[:, 0:1])

    # ---- main attention loop ----
    outT = big.tile([C, NQ], F32R)
    for ic in range(IC):
        i0 = 512 * ic
        pvh = [psum_acc.tile([128, 512], F32, tag="pv", name=f"pvh{h}")
               for h in range(HEADS)]
        for jc in range(JC):
            j0 = 128 * jc
            exps = []
            for pair in range(2):  # heads (0,1) then (2,3)
                st = psum.tile([128, 1024], F32, tag="st")
                ex = expp.tile([128, 1024], F32R, tag="ex")
                for hh in range(2):
                    h = 2 * pair + hh
                    hp = 32 * h
                    nc.tensor.matmul(
                        st[:, 512 * hh:512 * hh + 512],
                        kT[hp:hp + 32, j0:j0 + 128],
                        qTs[hp:hp + 32, i0:i0 + 512],
                        start=True, stop=True, tile_position=(hp, 0))
                nc.scalar.activation(ex[:], st[:], AF.Exp, scale=SCALE)
                exps.append(ex)
            for h in range(HEADS):
                ex = exps[h // 2][:, 512 * (h % 2):512 * (h % 2) + 512]
                blk = (h * JC + jc) * 128
                nc.tensor.matmul(
                    pvh[h][:, :],
                    v_aug[:, blk:blk + 128],
                    ex,
                    start=(jc == 0), stop=(jc == JC - 1))
        # normalize: outT[32h:32h+32, i] = numer / den_h
        rec = recp.tile([128, 2048], F32, tag="rec")
        recb = recp.tile([128, 512], F32, tag="recb")
        for h in range(HEADS):
            hp = 32 * h
            dr = (hp + 32) % 128
            nc.vector.reciprocal(rec[dr:dr + 1, 512 * h:512 * h + 512],
                                 pvh[h][dr:dr + 1, :])
            nc.sync.dma_start(out=recd[ic, h, :],
                              in_=rec[dr:dr + 1, 512 * h:512 * h + 512])
            dsrc = recd[ic, h, :]
            bcast = bass.AP(tensor=dsrc.tensor, offset=dsrc.offset,
                            ap=[[0, 32]] + list(dsrc.ap))
            nc.sync.dma_start(out=recb[hp:hp + 32, :], in_=bcast)
            nc.vector.tensor_mul(outT[hp:hp + 32, i0:i0 + 512],
                                 pvh[h][hp:hp + 32, :], recb[hp:hp + 32, :])
        # output projection for this query chunk: out_cT = w_out^T @ outT + b
        po = psum.tile([128, 512], F32, tag="st")
        nc.tensor.matmul(po[:, 0:512], wo[:], outT[:, i0:i0 + 512],
                         start=True, stop=True)
        nc.vector.tensor_scalar_add(res[:, i0:i0 + 512], po[:, 0:512],
                                    bias[:, 0:1])
        nc.sync.dma_start(out=out_d[:, i0:i0 + 512], in_=res[:, i0:i0 + 512])


_CACHE = {}


def build_program():
    if "nc" not in _CACHE:
        nc = bacc.Bacc("TRN2", debug=False, target_bir_lowering=False,
                       num_devices=N_CORES)
        with tile.TileContext(nc) as tc:
            _attention_kernel(tc)
        nc.compile()
        _CACHE["nc"] = nc
    return _CACHE["nc"]


def make_in_maps(x, w_qkv, w_out, b_out):
    in_maps = []
    for core in range(N_CORES):
        b, half = core // 2, core % 2
        i0 = half * NQ
        xr = np.asarray(x[b], dtype=np.float32).reshape(S, C)
        xT = np.ascontiguousarray(np.roll(xr, -i0, axis=0).T)
        in_maps.append({
            "xT": xT,
            "w_qkv": np.ascontiguousarray(w_qkv, dtype=np.float32),
            "w_out": np.ascontiguousarray(w_out, dtype=np.float32),
            "b_out": np.ascontiguousarray(b_out, dtype=np.float32).reshape(C, 1),
        })
    return in_maps


def assemble_output(per_core_outs):
    out = np.zeros((4, S, C), dtype=np.float32)
    for core, r in enumerate(per_core_outs):
        b, half = core // 2, core % 2
        out[b, half * NQ:(half + 1) * NQ] = np.asarray(r, dtype=np.float32).T
    return out.reshape(4, 64, 64, C)


def kernel(x, w_qkv, w_out, b_out):
    from concourse.bass_utils import run_bass_kernel_spmd
    nc = build_program()
    in_maps = make_in_maps(x, w_qkv, w_out, b_out)
    res = run_bass_kernel_spmd(nc, in_maps, list(range(N_CORES)))
    return assemble_output([r["out_cT"] for r in res.results])


if __name__ == "__main__":
    x = np.random.randn(4, 64, 64, C).astype(np.float32)
    w_qkv = (np.random.randn(C, 384) / np.sqrt(C)).astype(np.float32)
    w_out = (np.random.randn(C, C) / np.sqrt(C)).astype(np.float32)
    b_out = np.zeros(C, dtype=np.float32)
    out = kernel(x=x, w_qkv=w_qkv, w_out=w_out, b_out=b_out)
    print("kernel output", out.shape, out.dtype)
